# revision 16
# baseline (speedup 1.0000x reference)
"""Trainium2 Bass kernel for nn_CorrespondenceLoss.

Correspondence (hinge-margin descriptor) loss over B=8 images, data-parallel
across 8 NeuronCores (one image per core).

Per image (C=64 channels, H=W=64 grid, N=2048 correspondences):
  d1_all = normalize(f1.reshape(C, HW));  d2_all = normalize(f2.reshape(C, HW))
  d1 = d1_all[:, ids]; d2 = d2_all[:, lin(pos2)]
  positive[n] = 2 - 2 * <d1_n, d2_n>
  neg2[n] = min_m (2 - 2*<d1_n, d2_all_m> + 10*[cheb(pos2_n, m) <= 4])
  neg1[n] = min_m (2 - 2*<d2_n, d1_all_m> + 10*[cheb(pos1_n, m) <= 4])
  loss = mean relu(1 + positive - min(neg1, neg2))

Device strategy per image ("matrix" = one of the two N x HW distance matrices):
  The masked min over m is computed as a masked max over inner products.
  The Chebyshev ball is a row-window x col-window rectangle.  The column
  window is folded INTO the matmul with augmented contraction channels:
    innerQ[n, m] = <d1_n, d2_m> - 5 * [|c(m) - c_n| <= 4]
  via lhsT rows 64:128 = -5*cnear^T and rhs rows 64:128 = tile(I64, HW/64)
  (a -5 shift pushes any col-masked entry below every unmasked entry, since
  inner products of unit vectors lie in [-1, 1]).
  Per (anchor n, grid row r):
    P[n,r] = max_c innerP   (plain, K=64 matmul)
    Q[n,r] = max_c innerQ   (col-masked, K=128 matmul)
  and the row-window select is R = max(Q, P - 10*rnear[n,r]), then
  negInner[n] = max_r R.  Host combines:  loss_n = relu(1 - 2*posInner + 2*max(negInner1, negInner2)).

Host does only O(C*HW + N) prep: normalization scales, gathers by index,
mask/one-hot construction, and the final O(N) hinge+mean.
"""

import numpy as np

C = 64
H = 64
W = 64
HW = H * W
N = 2048
B = 8
NT = N // 128  # 16 primary anchor tiles per image (row-bucketed)
NSPILL = 2  # spill tiles for row-bucket overflow
NT2 = NT + NSPILL
NSLOT = NT2 * 128
MJ = HW // 512  # 8 matmul column blocks
SAFE = 4


def _tile_window(t):
    """Static grid-row window covering every safe-radius band of anchors
    whose row lies in bucket [4t, 4t+4)."""
    wlo = max(0, 4 * t - SAFE)
    whi = min(H, 4 * t + 4 + SAFE)
    return wlo, whi

_COMPILED = {}
LAST_EXEC_NS = None


# ---------------------------------------------------------------------------
# walrus in this environment accepts at most ONE sync-wait per instruction;
# Tile emits instructions with several.  Hoist extras onto NoOps inserted
# just before the over-subscribed instruction (same engine, so program order
# and the wait semantics are preserved).
# ---------------------------------------------------------------------------
def _split_multi_waits(nc, limit=1):
    import bass_rust
    from concourse import mybir

    ctr = 0
    for fn in nc.m.functions:
        for bb in fn.blocks:
            new = []
            for inst in bb.instructions:
                si = inst.sync_info
                if si is not None and len(si.on_wait) > limit:
                    waits = list(si.on_wait)
                    sem = [w for w in waits if w.sync_type == "semaphore"]
                    other = [w for w in waits if w.sync_type != "semaphore"]
                    keep_budget = max(0, limit - len(other))
                    move = sem[:-keep_budget] if keep_budget > 0 else sem
                    keep = other + (sem[-keep_budget:] if keep_budget > 0 else [])
                    if len(keep) > limit:
                        raise RuntimeError(
                            f"cannot split waits on {inst.name}: "
                            f"{len(other)} non-semaphore waits"
                        )
                    for w in move:
                        ctr += 1
                        new.append(
                            mybir.InstNoOp(
                                name=f"WSPLIT-{ctr}",
                                engine=inst.engine,
                                sync_info=bass_rust.SyncInfo(
                                    on_wait=[w], on_update=[]
                                ),
                            )
                        )
                    inst.sync_info = bass_rust.SyncInfo(
                        on_wait=keep, on_update=list(si.on_update)
                    )
                new.append(inst)
            bb.instructions = new
    return ctr


MM_DTYPE = "bfloat16"  # matmul operand dtype: "float32" or "bfloat16"


def _build_program():
    import concourse.bass as bass
    import concourse.tile as tile
    from concourse import mybir

    f32 = mybir.dt.float32
    mmdt = getattr(mybir.dt, MM_DTYPE)
    nc = bass.Bass()

    a2 = nc.dram_tensor("a2", [128, NSLOT], mmdt, kind="ExternalInput")
    r2 = nc.dram_tensor("r2", [128, HW], mmdt, kind="ExternalInput")
    a1 = nc.dram_tensor("a1", [128, NSLOT], mmdt, kind="ExternalInput")
    r1 = nc.dram_tensor("r1", [128, HW], mmdt, kind="ExternalInput")
    rn2 = nc.dram_tensor("rn2", [NSLOT, 64], f32, kind="ExternalInput")
    rn1 = nc.dram_tensor("rn1", [NSLOT, 64], f32, kind="ExternalInput")
    out2 = nc.dram_tensor("out2", [128, NT2], f32, kind="ExternalOutput")
    out1 = nc.dram_tensor("out1", [128, NT2], f32, kind="ExternalOutput")

    with tile.TileContext(nc) as tc:
        with (
            tc.tile_pool(name="singles", bufs=1) as singles,
            tc.tile_pool(name="small", bufs=4) as small,
            tc.tile_pool(name="outp", bufs=1) as outp,
            tc.tile_pool(name="ps", bufs=2, space="PSUM") as psum,
        ):
            a2_s = singles.tile([128, NSLOT], mmdt)
            r2_s = singles.tile([128, HW], mmdt)
            a1_s = singles.tile([128, NSLOT], mmdt)
            r1_s = singles.tile([128, HW], mmdt)
            nc.sync.dma_start(a2_s[:], a2[:])
            nc.sync.dma_start(r2_s[:], r2[:])
            nc.sync.dma_start(a1_s[:], a1[:])
            nc.sync.dma_start(r1_s[:], r1[:])
            out2_s = outp.tile([128, NT2], f32)
            out1_s = outp.tile([128, NT2], f32)

            for a_s, r_s, rn, out_s in (
                (a2_s, r2_s, rn2, out2_s),
                (a1_s, r1_s, rn1, out1_s),
            ):
                for t in range(NT2):
                    aslc = slice(t * 128, (t + 1) * 128)
                    p_t = small.tile([128, 64], f32, tag="p")
                    # P variant (plain, K=64) over the full grid
                    for h in range(2):  # m-halves; half h covers r in [32h, 32h+32)
                        ps_t = psum.tile([128, HW // 2], f32, tag="ps")
                        for j in range(MJ // 2):
                            mslc = slice(
                                h * (HW // 2) + j * 512,
                                h * (HW // 2) + (j + 1) * 512,
                            )
                            nc.tensor.matmul(
                                ps_t[:, j * 512 : (j + 1) * 512],
                                a_s[0:64, aslc],
                                r_s[0:64, mslc],
                                start=True,
                                stop=True,
                            )
                        nc.vector.tensor_reduce(
                            p_t[:, h * 32 : (h + 1) * 32],
                            ps_t[:].rearrange("p (r c) -> p r c", c=64),
                            axis=mybir.AxisListType.X,
                            op=mybir.AluOpType.max,
                        )
                    # Q variant (col-masked, K=128): primary tiles only need
                    # the static 12-grid-row window; spill tiles need all 64.
                    if t < NT:
                        wlo, whi = _tile_window(t)
                    else:
                        wlo, whi = 0, H
                    q_t = small.tile([128, whi - wlo], f32, tag="q")
                    ncols = (whi - wlo) * 64
                    for q0 in range(0, ncols, 2048):
                        qw = min(2048, ncols - q0)
                        ps_q = psum.tile([128, qw], f32, tag="ps")
                        for j in range(0, qw, 512):
                            jw = min(512, qw - j)
                            mslc = slice(
                                wlo * 64 + q0 + j, wlo * 64 + q0 + j + jw
                            )
                            nc.tensor.matmul(
                                ps_q[:, j : j + jw],
                                a_s[:, aslc],
                                r_s[:, mslc],
                                start=True,
                                stop=True,
                            )
                        nc.vector.tensor_reduce(
                            q_t[:, q0 // 64 : (q0 + qw) // 64],
                            ps_q[:].rearrange("p (r c) -> p r c", c=64),
                            axis=mybir.AxisListType.X,
                            op=mybir.AluOpType.max,
                        )
                    rn_t = small.tile([128, 64], f32, tag="rn")
                    nc.sync.dma_start(rn_t[:], rn[aslc, :])
                    # sel = P - 10*rnear; sel[win] = max(sel[win], Q); negInner = max_r sel
                    sel = small.tile([128, 64], f32, tag="sel")
                    nc.vector.tensor_sub(sel[:], p_t[:], rn_t[:])
                    nc.vector.tensor_max(
                        sel[:, wlo:whi], sel[:, wlo:whi], q_t[:]
                    )
                    nc.vector.reduce_max(
                        out_s[:, t : t + 1], sel[:], axis=mybir.AxisListType.X
                    )

            nc.sync.dma_start(out2[:], out2_s[:])
            nc.sync.dma_start(out1[:], out1_s[:])

    return nc


def _assign_slots(rv):
    """Bucket anchors by grid row into NT primary tiles (rows [4t, 4t+4))
    plus NSPILL overflow tiles.  Returns (perm [NSLOT], valid [NSLOT])."""
    spill = []
    perm = np.zeros(NSLOT, dtype=np.int64)
    valid = np.zeros(NSLOT, dtype=bool)
    for t in range(NT):
        b = np.where((rv >= 4 * t) & (rv < 4 * t + 4))[0]
        take = b[:128]
        spill.extend(b[128:].tolist())
        perm[t * 128 : t * 128 + len(take)] = take
        valid[t * 128 : t * 128 + len(take)] = True
        if len(take) < 128 and len(take) > 0:
            perm[t * 128 + len(take) : (t + 1) * 128] = take[0]
    if len(spill) > NSPILL * 128:
        raise RuntimeError(f"row-bucket spill overflow: {len(spill)}")
    s0 = NT * 128
    perm[s0 : s0 + len(spill)] = spill
    valid[s0 : s0 + len(spill)] = True
    return perm, valid


def _prep_image(f1, f2, idv, r2v, c2v):
    """Host-side index/mask prep for one image."""
    f1 = f1.reshape(C, HW)
    f2 = f2.reshape(C, HW)
    n1 = np.sqrt((f1 * f1).sum(axis=0))
    f1n = f1 / np.maximum(n1, 1e-12)
    n2 = np.sqrt((f2 * f2).sum(axis=0))
    f2n = f2 / np.maximum(n2, 1e-12)

    r1v = idv // W
    c1v = idv % W
    lin2 = r2v * W + c2v

    d1n = f1n[:, idv]  # [C, N]
    d2n = f2n[:, lin2]  # [C, N]
    pos_inner = (d1n * d2n).sum(axis=0)  # [N]

    perm2, valid2 = _assign_slots(r2v)
    perm1, valid1 = _assign_slots(r1v)

    w = np.arange(64)
    c2p = c2v[perm2]
    c1p = c1v[perm1]
    cn2 = -5.0 * (np.abs(w[:, None] - c2p[None, :]) <= SAFE)  # [64, NSLOT]
    cn1 = -5.0 * (np.abs(w[:, None] - c1p[None, :]) <= SAFE)
    rn2 = 10.0 * (np.abs(w[None, :] - r2v[perm2][:, None]) <= SAFE)  # [NSLOT, 64]
    rn1 = 10.0 * (np.abs(w[None, :] - r1v[perm1][:, None]) <= SAFE)

    onehot = np.tile(np.eye(64, dtype=np.float32), (1, HW // 64))  # [64, HW]

    if MM_DTYPE == "bfloat16":
        from ml_dtypes import bfloat16 as mmdt
    else:
        mmdt = np.float32
    f32 = np.float32
    return {
        "a2": np.concatenate([d1n[:, perm2], cn2], axis=0).astype(mmdt),
        "r2": np.concatenate([f2n, onehot], axis=0).astype(mmdt),
        "a1": np.concatenate([d2n[:, perm1], cn1], axis=0).astype(mmdt),
        "r1": np.concatenate([f1n, onehot], axis=0).astype(mmdt),
        "rn2": rn2.astype(f32),
        "rn1": rn1.astype(f32),
    }, pos_inner.astype(f32), (perm2, valid2, perm1, valid1)


def kernel(x1_encoded, x2_encoded, ids, fmap_pos2, trace=False):
    global LAST_EXEC_NS
    from concourse.bass_utils import run_bass_kernel_spmd

    x1 = np.asarray(x1_encoded, dtype=np.float32)
    x2 = np.asarray(x2_encoded, dtype=np.float32)
    idsv = np.asarray(ids)
    pos2 = np.asarray(fmap_pos2)

    in_maps = []
    pos_inner = []
    perms = []
    for b in range(B):
        m, pi, pv = _prep_image(
            x1[b], x2[b], idsv[b].astype(np.int64),
            pos2[b, 0].astype(np.int64), pos2[b, 1].astype(np.int64),
        )
        in_maps.append(m)
        pos_inner.append(pi)
        perms.append(pv)

    if "nc" not in _COMPILED:
        nc = _build_program()
        _split_multi_waits(nc)
        _COMPILED["nc"] = nc
    nc = _COMPILED["nc"]

    if trace:
        _install_profile_hook()
    res = run_bass_kernel_spmd(
        nc, in_maps, core_ids=list(range(B)), trace=trace
    )
    if trace:
        LAST_EXEC_NS = res.exec_time_ns

    per_image = np.empty(B, dtype=np.float32)
    for b in range(B):
        perm2, valid2, perm1, valid1 = perms[b]
        v2 = res.results[b]["out2"].T.reshape(-1)  # [NSLOT] (slot = t*128 + p)
        v1 = res.results[b]["out1"].T.reshape(-1)
        neg_in2 = np.empty(N, dtype=np.float32)
        neg_in1 = np.empty(N, dtype=np.float32)
        neg_in2[perm2[valid2]] = v2[valid2]
        neg_in1[perm1[valid1]] = v1[valid1]
        max_inner = np.maximum(neg_in1, neg_in2)
        loss_n = np.maximum(1.0 - 2.0 * pos_inner[b] + 2.0 * max_inner, 0.0)
        per_image[b] = loss_n.mean(dtype=np.float64)
    return np.array(per_image.mean(dtype=np.float64), dtype=np.float32)


def _install_profile_hook():
    """antenv.axon_hooks is absent on this image; synthesize it so
    run_bass_kernel_spmd(trace=True) can capture NTFF profiles."""
    import sys
    import types

    if "antenv.axon_hooks" in sys.modules:
        return
    mod = types.ModuleType("antenv.axon_hooks")
    mod._hook = None
    mod.set_axon_ntff_profile_hook = lambda h: setattr(mod, "_hook", h)
    mod.get_axon_ntff_profile_hook = lambda: mod._hook
    sys.modules["antenv.axon_hooks"] = mod
    try:
        import antenv

        antenv.axon_hooks = mod
        from trn_agent_boot.trn_boot import _ntff_profile_via_ctypes

        hook = _ntff_profile_via_ctypes("/opt/axon/libaxon_pjrt.so")
        if hook is not None:
            mod.set_axon_ntff_profile_hook(hook)
    except Exception:
        pass


# revision 19
# speedup vs baseline: 1.1181x; 1.1181x over previous
"""Trainium2 Bass kernel for nn_CorrespondenceLoss.

Correspondence (hinge-margin descriptor) loss over B=8 images, data-parallel
across 8 NeuronCores (one image per core).

Per image (C=64 channels, H=W=64 grid, N=2048 correspondences):
  d1_all = normalize(f1.reshape(C, HW));  d2_all = normalize(f2.reshape(C, HW))
  d1 = d1_all[:, ids]; d2 = d2_all[:, lin(pos2)]
  positive[n] = 2 - 2 * <d1_n, d2_n>
  neg2[n] = min_m (2 - 2*<d1_n, d2_all_m> + 10*[cheb(pos2_n, m) <= 4])
  neg1[n] = min_m (2 - 2*<d2_n, d1_all_m> + 10*[cheb(pos1_n, m) <= 4])
  loss = mean relu(1 + positive - min(neg1, neg2))

Device strategy per image ("matrix" = one of the two N x HW distance matrices):
  The masked min over m is computed as a masked max over inner products.
  The Chebyshev ball is a row-window x col-window rectangle.  The column
  window is folded INTO the matmul with augmented contraction channels:
    innerQ[n, m] = <d1_n, d2_m> - 5 * [|c(m) - c_n| <= 4]
  via lhsT rows 64:128 = -5*cnear^T and rhs rows 64:128 = tile(I64, HW/64)
  (a -5 shift pushes any col-masked entry below every unmasked entry, since
  inner products of unit vectors lie in [-1, 1]).
  Per (anchor n, grid row r):
    P[n,r] = max_c innerP   (plain, K=64 matmul)
    Q[n,r] = max_c innerQ   (col-masked, K=128 matmul)
  and the row-window select is R = max(Q, P - 10*rnear[n,r]), then
  negInner[n] = max_r R.  Host combines:  loss_n = relu(1 - 2*posInner + 2*max(negInner1, negInner2)).

Host does only O(C*HW + N) prep: normalization scales, gathers by index,
mask/one-hot construction, and the final O(N) hinge+mean.
"""

import numpy as np

C = 64
H = 64
W = 64
HW = H * W
N = 2048
B = 8
NT = N // 128  # 16 primary anchor tiles per image (row-bucketed)
NSPILL = 2  # spill tiles for row-bucket overflow
NT2 = NT + NSPILL
NSLOT = NT2 * 128
MJ = HW // 512  # 8 matmul column blocks
SAFE = 4


def _tile_window(t):
    """Static grid-row window covering every safe-radius band of anchors
    whose row lies in bucket [4t, 4t+4)."""
    wlo = max(0, 4 * t - SAFE)
    whi = min(H, 4 * t + 4 + SAFE)
    return wlo, whi

_COMPILED = {}
LAST_EXEC_NS = None


# ---------------------------------------------------------------------------
# walrus in this environment accepts at most ONE sync-wait per instruction;
# Tile emits instructions with several.  Hoist extras onto NoOps inserted
# just before the over-subscribed instruction (same engine, so program order
# and the wait semantics are preserved).
# ---------------------------------------------------------------------------
def _split_multi_waits(nc, limit=1):
    import bass_rust
    from concourse import mybir

    ctr = 0
    for fn in nc.m.functions:
        for bb in fn.blocks:
            new = []
            for inst in bb.instructions:
                si = inst.sync_info
                if si is not None and len(si.on_wait) > limit:
                    waits = list(si.on_wait)
                    sem = [w for w in waits if w.sync_type == "semaphore"]
                    other = [w for w in waits if w.sync_type != "semaphore"]
                    keep_budget = max(0, limit - len(other))
                    move = sem[:-keep_budget] if keep_budget > 0 else sem
                    keep = other + (sem[-keep_budget:] if keep_budget > 0 else [])
                    if len(keep) > limit:
                        raise RuntimeError(
                            f"cannot split waits on {inst.name}: "
                            f"{len(other)} non-semaphore waits"
                        )
                    for w in move:
                        ctr += 1
                        new.append(
                            mybir.InstNoOp(
                                name=f"WSPLIT-{ctr}",
                                engine=inst.engine,
                                sync_info=bass_rust.SyncInfo(
                                    on_wait=[w], on_update=[]
                                ),
                            )
                        )
                    inst.sync_info = bass_rust.SyncInfo(
                        on_wait=keep, on_update=list(si.on_update)
                    )
                new.append(inst)
            bb.instructions = new
    return ctr


MM_DTYPE = "bfloat16"  # matmul operand dtype: "float32" or "bfloat16"


def _build_program():
    import concourse.bass as bass
    import concourse.tile as tile
    from concourse import mybir

    f32 = mybir.dt.float32
    mmdt = getattr(mybir.dt, MM_DTYPE)
    nc = bass.Bass()

    a2 = nc.dram_tensor("a2", [128, NSLOT], mmdt, kind="ExternalInput")
    r2 = nc.dram_tensor("r2", [128, HW], mmdt, kind="ExternalInput")
    a1 = nc.dram_tensor("a1", [128, NSLOT], mmdt, kind="ExternalInput")
    r1 = nc.dram_tensor("r1", [128, HW], mmdt, kind="ExternalInput")
    rn2 = nc.dram_tensor("rn2", [NSLOT, 64], f32, kind="ExternalInput")
    rn1 = nc.dram_tensor("rn1", [NSLOT, 64], f32, kind="ExternalInput")
    out2 = nc.dram_tensor("out2", [128, NT2], f32, kind="ExternalOutput")
    out1 = nc.dram_tensor("out1", [128, NT2], f32, kind="ExternalOutput")

    bf16 = mybir.dt.bfloat16

    with tile.TileContext(nc) as tc:
        with (
            tc.tile_pool(name="singles", bufs=1) as singles,
            tc.tile_pool(name="small", bufs=4) as small,
            tc.tile_pool(name="tree", bufs=3) as tree,
            tc.tile_pool(name="outp", bufs=1) as outp,
            tc.tile_pool(name="ps", bufs=2, space="PSUM") as psum,
        ):

            def reduce_chunk_act(ps_t, dst):
                """max over innermost 64 of a [128, 2048] PSUM chunk ->
                dst [128, 32], splitting work ACT (cast) + DVE (2x tree)."""
                cast = tree.tile([128, 32, 64], bf16, tag="cast")
                nc.scalar.copy(
                    cast[:], ps_t[:].rearrange("p (r c) -> p r c", c=64)
                )
                t1 = tree.tile([128, 32, 32], bf16, tag="t1")
                nc.vector.tensor_max(t1[:], cast[:, :, 0:32], cast[:, :, 32:64])
                t2 = tree.tile([128, 32, 16], bf16, tag="t2")
                nc.vector.tensor_max(t2[:], t1[:, :, 0:16], t1[:, :, 16:32])
                t3 = tree.tile([128, 32, 8], bf16, tag="t3")
                nc.vector.tensor_max(t3[:], t2[:, :, 0:8], t2[:, :, 8:16])
                nc.vector.tensor_reduce(
                    dst,
                    t3[:],
                    axis=mybir.AxisListType.X,
                    op=mybir.AluOpType.max,
                )
            a2_s = singles.tile([128, NSLOT], mmdt)
            r2_s = singles.tile([128, HW], mmdt)
            a1_s = singles.tile([128, NSLOT], mmdt)
            r1_s = singles.tile([128, HW], mmdt)
            nc.sync.dma_start(a2_s[:], a2[:])
            nc.sync.dma_start(r2_s[:], r2[:])
            nc.sync.dma_start(a1_s[:], a1[:])
            nc.sync.dma_start(r1_s[:], r1[:])
            out2_s = outp.tile([128, NT2], f32)
            out1_s = outp.tile([128, NT2], f32)

            for a_s, r_s, rn, out_s in (
                (a2_s, r2_s, rn2, out2_s),
                (a1_s, r1_s, rn1, out1_s),
            ):
                for t in range(NT2):
                    aslc = slice(t * 128, (t + 1) * 128)
                    p_t = small.tile([128, 64], f32, tag="p")
                    # P variant (plain, K=64) over the full grid
                    for h in range(2):  # m-halves; half h covers r in [32h, 32h+32)
                        ps_t = psum.tile([128, HW // 2], f32, tag="ps")
                        for j in range(MJ // 2):
                            mslc = slice(
                                h * (HW // 2) + j * 512,
                                h * (HW // 2) + (j + 1) * 512,
                            )
                            nc.tensor.matmul(
                                ps_t[:, j * 512 : (j + 1) * 512],
                                a_s[0:64, aslc],
                                r_s[0:64, mslc],
                                start=True,
                                stop=True,
                            )
                        reduce_chunk_act(ps_t, p_t[:, h * 32 : (h + 1) * 32])
                    # Q variant (col-masked, K=128): primary tiles only need
                    # the static 12-grid-row window; spill tiles need all 64.
                    if t < NT:
                        wlo, whi = _tile_window(t)
                    else:
                        wlo, whi = 0, H
                    q_t = small.tile([128, whi - wlo], f32, tag="q")
                    ncols = (whi - wlo) * 64
                    for q0 in range(0, ncols, 2048):
                        qw = min(2048, ncols - q0)
                        ps_q = psum.tile([128, qw], f32, tag="ps")
                        for j in range(0, qw, 512):
                            jw = min(512, qw - j)
                            mslc = slice(
                                wlo * 64 + q0 + j, wlo * 64 + q0 + j + jw
                            )
                            nc.tensor.matmul(
                                ps_q[:, j : j + jw],
                                a_s[:, aslc],
                                r_s[:, mslc],
                                start=True,
                                stop=True,
                            )
                        if qw == 2048:
                            reduce_chunk_act(
                                ps_q, q_t[:, q0 // 64 : (q0 + qw) // 64]
                            )
                        else:
                            nc.vector.tensor_reduce(
                                q_t[:, q0 // 64 : (q0 + qw) // 64],
                                ps_q[:].rearrange("p (r c) -> p r c", c=64),
                                axis=mybir.AxisListType.X,
                                op=mybir.AluOpType.max,
                            )
                    rn_t = small.tile([128, 64], f32, tag="rn")
                    nc.sync.dma_start(rn_t[:], rn[aslc, :])
                    # sel = P - 10*rnear; sel[win] = max(sel[win], Q); negInner = max_r sel
                    sel = small.tile([128, 64], f32, tag="sel")
                    nc.vector.tensor_sub(sel[:], p_t[:], rn_t[:])
                    nc.vector.tensor_max(
                        sel[:, wlo:whi], sel[:, wlo:whi], q_t[:]
                    )
                    nc.vector.reduce_max(
                        out_s[:, t : t + 1], sel[:], axis=mybir.AxisListType.X
                    )

            nc.sync.dma_start(out2[:], out2_s[:])
            nc.sync.dma_start(out1[:], out1_s[:])

    return nc


def _assign_slots(rv):
    """Bucket anchors by grid row into NT primary tiles (rows [4t, 4t+4))
    plus NSPILL overflow tiles.  Returns (perm [NSLOT], valid [NSLOT])."""
    spill = []
    perm = np.zeros(NSLOT, dtype=np.int64)
    valid = np.zeros(NSLOT, dtype=bool)
    for t in range(NT):
        b = np.where((rv >= 4 * t) & (rv < 4 * t + 4))[0]
        take = b[:128]
        spill.extend(b[128:].tolist())
        perm[t * 128 : t * 128 + len(take)] = take
        valid[t * 128 : t * 128 + len(take)] = True
        if len(take) < 128 and len(take) > 0:
            perm[t * 128 + len(take) : (t + 1) * 128] = take[0]
    if len(spill) > NSPILL * 128:
        raise RuntimeError(f"row-bucket spill overflow: {len(spill)}")
    s0 = NT * 128
    perm[s0 : s0 + len(spill)] = spill
    valid[s0 : s0 + len(spill)] = True
    return perm, valid


def _prep_image(f1, f2, idv, r2v, c2v):
    """Host-side index/mask prep for one image."""
    f1 = f1.reshape(C, HW)
    f2 = f2.reshape(C, HW)
    n1 = np.sqrt((f1 * f1).sum(axis=0))
    f1n = f1 / np.maximum(n1, 1e-12)
    n2 = np.sqrt((f2 * f2).sum(axis=0))
    f2n = f2 / np.maximum(n2, 1e-12)

    r1v = idv // W
    c1v = idv % W
    lin2 = r2v * W + c2v

    d1n = f1n[:, idv]  # [C, N]
    d2n = f2n[:, lin2]  # [C, N]
    pos_inner = (d1n * d2n).sum(axis=0)  # [N]

    perm2, valid2 = _assign_slots(r2v)
    perm1, valid1 = _assign_slots(r1v)

    w = np.arange(64)
    c2p = c2v[perm2]
    c1p = c1v[perm1]
    cn2 = -5.0 * (np.abs(w[:, None] - c2p[None, :]) <= SAFE)  # [64, NSLOT]
    cn1 = -5.0 * (np.abs(w[:, None] - c1p[None, :]) <= SAFE)
    rn2 = 10.0 * (np.abs(w[None, :] - r2v[perm2][:, None]) <= SAFE)  # [NSLOT, 64]
    rn1 = 10.0 * (np.abs(w[None, :] - r1v[perm1][:, None]) <= SAFE)

    onehot = np.tile(np.eye(64, dtype=np.float32), (1, HW // 64))  # [64, HW]

    if MM_DTYPE == "bfloat16":
        from ml_dtypes import bfloat16 as mmdt
    else:
        mmdt = np.float32
    f32 = np.float32
    return {
        "a2": np.concatenate([d1n[:, perm2], cn2], axis=0).astype(mmdt),
        "r2": np.concatenate([f2n, onehot], axis=0).astype(mmdt),
        "a1": np.concatenate([d2n[:, perm1], cn1], axis=0).astype(mmdt),
        "r1": np.concatenate([f1n, onehot], axis=0).astype(mmdt),
        "rn2": rn2.astype(f32),
        "rn1": rn1.astype(f32),
    }, pos_inner.astype(f32), (perm2, valid2, perm1, valid1)


def kernel(x1_encoded, x2_encoded, ids, fmap_pos2, trace=False):
    global LAST_EXEC_NS
    from concourse.bass_utils import run_bass_kernel_spmd

    x1 = np.asarray(x1_encoded, dtype=np.float32)
    x2 = np.asarray(x2_encoded, dtype=np.float32)
    idsv = np.asarray(ids)
    pos2 = np.asarray(fmap_pos2)

    in_maps = []
    pos_inner = []
    perms = []
    for b in range(B):
        m, pi, pv = _prep_image(
            x1[b], x2[b], idsv[b].astype(np.int64),
            pos2[b, 0].astype(np.int64), pos2[b, 1].astype(np.int64),
        )
        in_maps.append(m)
        pos_inner.append(pi)
        perms.append(pv)

    if "nc" not in _COMPILED:
        nc = _build_program()
        _split_multi_waits(nc)
        _COMPILED["nc"] = nc
    nc = _COMPILED["nc"]

    if trace:
        _install_profile_hook()
    res = run_bass_kernel_spmd(
        nc, in_maps, core_ids=list(range(B)), trace=trace
    )
    if trace:
        LAST_EXEC_NS = res.exec_time_ns

    per_image = np.empty(B, dtype=np.float32)
    for b in range(B):
        perm2, valid2, perm1, valid1 = perms[b]
        v2 = res.results[b]["out2"].T.reshape(-1)  # [NSLOT] (slot = t*128 + p)
        v1 = res.results[b]["out1"].T.reshape(-1)
        neg_in2 = np.empty(N, dtype=np.float32)
        neg_in1 = np.empty(N, dtype=np.float32)
        neg_in2[perm2[valid2]] = v2[valid2]
        neg_in1[perm1[valid1]] = v1[valid1]
        max_inner = np.maximum(neg_in1, neg_in2)
        loss_n = np.maximum(1.0 - 2.0 * pos_inner[b] + 2.0 * max_inner, 0.0)
        per_image[b] = loss_n.mean(dtype=np.float64)
    return np.array(per_image.mean(dtype=np.float64), dtype=np.float32)


def _install_profile_hook():
    """antenv.axon_hooks is absent on this image; synthesize it so
    run_bass_kernel_spmd(trace=True) can capture NTFF profiles."""
    import sys
    import types

    if "antenv.axon_hooks" in sys.modules:
        return
    mod = types.ModuleType("antenv.axon_hooks")
    mod._hook = None
    mod.set_axon_ntff_profile_hook = lambda h: setattr(mod, "_hook", h)
    mod.get_axon_ntff_profile_hook = lambda: mod._hook
    sys.modules["antenv.axon_hooks"] = mod
    try:
        import antenv

        antenv.axon_hooks = mod
        from trn_agent_boot.trn_boot import _ntff_profile_via_ctypes

        hook = _ntff_profile_via_ctypes("/opt/axon/libaxon_pjrt.so")
        if hook is not None:
            mod.set_axon_ntff_profile_hook(hook)
    except Exception:
        pass


# revision 22
# speedup vs baseline: 1.1698x; 1.0463x over previous
"""Trainium2 Bass kernel for nn_CorrespondenceLoss.

Correspondence (hinge-margin descriptor) loss over B=8 images, data-parallel
across 8 NeuronCores (one image per core).

Per image (C=64 channels, H=W=64 grid, N=2048 correspondences):
  d1_all = normalize(f1.reshape(C, HW));  d2_all = normalize(f2.reshape(C, HW))
  d1 = d1_all[:, ids]; d2 = d2_all[:, lin(pos2)]
  positive[n] = 2 - 2 * <d1_n, d2_n>
  neg2[n] = min_m (2 - 2*<d1_n, d2_all_m> + 10*[cheb(pos2_n, m) <= 4])
  neg1[n] = min_m (2 - 2*<d2_n, d1_all_m> + 10*[cheb(pos1_n, m) <= 4])
  loss = mean relu(1 + positive - min(neg1, neg2))

Device strategy per image ("matrix" = one of the two N x HW distance matrices):
  The masked min over m is computed as a masked max over inner products.
  The Chebyshev ball is a row-window x col-window rectangle.  The column
  window is folded INTO the matmul with augmented contraction channels:
    innerQ[n, m] = <d1_n, d2_m> - 5 * [|c(m) - c_n| <= 4]
  via lhsT rows 64:128 = -5*cnear^T and rhs rows 64:128 = tile(I64, HW/64)
  (a -5 shift pushes any col-masked entry below every unmasked entry, since
  inner products of unit vectors lie in [-1, 1]).
  Per (anchor n, grid row r):
    P[n,r] = max_c innerP   (plain, K=64 matmul)
    Q[n,r] = max_c innerQ   (col-masked, K=128 matmul)
  and the row-window select is R = max(Q, P - 10*rnear[n,r]), then
  negInner[n] = max_r R.  Host combines:  loss_n = relu(1 - 2*posInner + 2*max(negInner1, negInner2)).

Host does only O(C*HW + N) prep: normalization scales, gathers by index,
mask/one-hot construction, and the final O(N) hinge+mean.
"""

import numpy as np

C = 64
H = 64
W = 64
HW = H * W
N = 2048
B = 8
NT = N // 128  # 16 primary anchor tiles per image (row-bucketed)
NSPILL = 2  # spill tiles for row-bucket overflow
NT2 = NT + NSPILL
NSLOT = NT2 * 128
MJ = HW // 512  # 8 matmul column blocks
SAFE = 4


def _tile_window(t):
    """Static grid-row window covering every safe-radius band of anchors
    whose row lies in bucket [4t, 4t+4)."""
    wlo = max(0, 4 * t - SAFE)
    whi = min(H, 4 * t + 4 + SAFE)
    return wlo, whi

_COMPILED = {}
LAST_EXEC_NS = None


# ---------------------------------------------------------------------------
# walrus in this environment accepts at most ONE sync-wait per instruction;
# Tile emits instructions with several.  Hoist extras onto NoOps inserted
# just before the over-subscribed instruction (same engine, so program order
# and the wait semantics are preserved).
# ---------------------------------------------------------------------------
def _split_multi_waits(nc, limit=1):
    import bass_rust
    from concourse import mybir

    ctr = 0
    for fn in nc.m.functions:
        for bb in fn.blocks:
            new = []
            for inst in bb.instructions:
                si = inst.sync_info
                if si is not None and len(si.on_wait) > limit:
                    waits = list(si.on_wait)
                    sem = [w for w in waits if w.sync_type == "semaphore"]
                    other = [w for w in waits if w.sync_type != "semaphore"]
                    keep_budget = max(0, limit - len(other))
                    move = sem[:-keep_budget] if keep_budget > 0 else sem
                    keep = other + (sem[-keep_budget:] if keep_budget > 0 else [])
                    if len(keep) > limit:
                        raise RuntimeError(
                            f"cannot split waits on {inst.name}: "
                            f"{len(other)} non-semaphore waits"
                        )
                    for w in move:
                        ctr += 1
                        new.append(
                            mybir.InstNoOp(
                                name=f"WSPLIT-{ctr}",
                                engine=inst.engine,
                                sync_info=bass_rust.SyncInfo(
                                    on_wait=[w], on_update=[]
                                ),
                            )
                        )
                    inst.sync_info = bass_rust.SyncInfo(
                        on_wait=keep, on_update=list(si.on_update)
                    )
                new.append(inst)
            bb.instructions = new
    return ctr


MM_DTYPE = "bfloat16"  # matmul operand dtype: "float32" or "bfloat16"


def _build_program():
    import concourse.bass as bass
    import concourse.tile as tile
    from concourse import mybir

    f32 = mybir.dt.float32
    mmdt = getattr(mybir.dt, MM_DTYPE)
    nc = bass.Bass()

    a2 = nc.dram_tensor("a2", [128, NSLOT], mmdt, kind="ExternalInput")
    r2 = nc.dram_tensor("r2", [128, HW], mmdt, kind="ExternalInput")
    a1 = nc.dram_tensor("a1", [128, NSLOT], mmdt, kind="ExternalInput")
    r1 = nc.dram_tensor("r1", [128, HW], mmdt, kind="ExternalInput")
    rn2 = nc.dram_tensor("rn2", [NSLOT, 64], f32, kind="ExternalInput")
    rn1 = nc.dram_tensor("rn1", [NSLOT, 64], f32, kind="ExternalInput")
    out2 = nc.dram_tensor("out2", [128, NT2], f32, kind="ExternalOutput")
    out1 = nc.dram_tensor("out1", [128, NT2], f32, kind="ExternalOutput")

    bf16 = mybir.dt.bfloat16

    with tile.TileContext(nc) as tc:
        with (
            tc.tile_pool(name="singles", bufs=1) as singles,
            tc.tile_pool(name="small", bufs=4) as small,
            tc.tile_pool(name="tree", bufs=3) as tree,
            tc.tile_pool(name="outp", bufs=1) as outp,
            tc.tile_pool(name="ps", bufs=2, space="PSUM") as psum,
        ):

            def reduce_chunk_act(ps_t, dst):
                """max over innermost 64 of a [128, 2048] PSUM chunk ->
                dst [128, 32], splitting work ACT (cast) + DVE (2x tree)."""
                cast = tree.tile([128, 32, 64], bf16, tag="cast")
                nc.scalar.copy(
                    cast[:], ps_t[:].rearrange("p (r c) -> p r c", c=64)
                )
                t1 = tree.tile([128, 32, 32], bf16, tag="t1")
                nc.vector.tensor_max(t1[:], cast[:, :, 0:32], cast[:, :, 32:64])
                t2 = tree.tile([128, 32, 16], bf16, tag="t2")
                nc.vector.tensor_max(t2[:], t1[:, :, 0:16], t1[:, :, 16:32])
                t3 = tree.tile([128, 32, 8], bf16, tag="t3")
                nc.vector.tensor_max(t3[:], t2[:, :, 0:8], t2[:, :, 8:16])
                nc.vector.tensor_reduce(
                    dst,
                    t3[:],
                    axis=mybir.AxisListType.X,
                    op=mybir.AluOpType.max,
                )
            a2_s = singles.tile([128, NSLOT], mmdt)
            r2_s = singles.tile([128, HW], mmdt)
            a1_s = singles.tile([128, NSLOT], mmdt)
            r1_s = singles.tile([128, HW], mmdt)
            nc.sync.dma_start(a2_s[:], a2[:])
            nc.sync.dma_start(r2_s[:], r2[:])
            nc.sync.dma_start(a1_s[:], a1[:])
            nc.sync.dma_start(r1_s[:], r1[:])
            # anchor/target duplicates in partitions 64:128 so pairs of K=64
            # P-matmuls can row-pack the PE array (tile_position rows 0/64)
            a2_d = singles.tile([128, NSLOT], mmdt)
            r2_d = singles.tile([128, HW], mmdt)
            a1_d = singles.tile([128, NSLOT], mmdt)
            r1_d = singles.tile([128, HW], mmdt)
            for dup, src in ((a2_d, a2), (r2_d, r2), (a1_d, a1), (r1_d, r1)):
                nc.sync.dma_start(dup[0:64, :], src[0:64, :])
                nc.sync.dma_start(dup[64:128, :], src[0:64, :])
            out2_s = outp.tile([128, NT2], f32)
            out1_s = outp.tile([128, NT2], f32)

            for a_s, r_s, a_d, r_d, rn, out_s in (
                (a2_s, r2_s, a2_d, r2_d, rn2, out2_s),
                (a1_s, r1_s, a1_d, r1_d, rn1, out1_s),
            ):
                for t in range(NT2):
                    aslc = slice(t * 128, (t + 1) * 128)
                    p_t = small.tile([128, 64], f32, tag="p")
                    # P variant (plain, K=64) over the full grid; pairs of
                    # blocks run concurrently in PE rows 0:64 / 64:128
                    for h in range(2):  # m-halves; half h covers r in [32h, 32h+32)
                        ps_t = psum.tile([128, HW // 2], f32, tag="ps")
                        for j in range(MJ // 2):
                            base = 64 * (j % 2)
                            mslc = slice(
                                h * (HW // 2) + j * 512,
                                h * (HW // 2) + (j + 1) * 512,
                            )
                            nc.tensor.matmul(
                                ps_t[:, j * 512 : (j + 1) * 512],
                                a_d[base : base + 64, aslc],
                                r_d[base : base + 64, mslc],
                                start=True,
                                stop=True,
                            )
                        reduce_chunk_act(ps_t, p_t[:, h * 32 : (h + 1) * 32])
                    # Q variant (col-masked, K=128): primary tiles only need
                    # the static 12-grid-row window; spill tiles need all 64.
                    if t < NT:
                        wlo, whi = _tile_window(t)
                    else:
                        wlo, whi = 0, H
                    q_t = small.tile([128, whi - wlo], f32, tag="q")
                    ncols = (whi - wlo) * 64
                    for q0 in range(0, ncols, 2048):
                        qw = min(2048, ncols - q0)
                        ps_q = psum.tile([128, qw], f32, tag="ps")
                        for j in range(0, qw, 512):
                            jw = min(512, qw - j)
                            mslc = slice(
                                wlo * 64 + q0 + j, wlo * 64 + q0 + j + jw
                            )
                            nc.tensor.matmul(
                                ps_q[:, j : j + jw],
                                a_s[:, aslc],
                                r_s[:, mslc],
                                start=True,
                                stop=True,
                            )
                        if qw == 2048:
                            reduce_chunk_act(
                                ps_q, q_t[:, q0 // 64 : (q0 + qw) // 64]
                            )
                        else:
                            nc.vector.tensor_reduce(
                                q_t[:, q0 // 64 : (q0 + qw) // 64],
                                ps_q[:].rearrange("p (r c) -> p r c", c=64),
                                axis=mybir.AxisListType.X,
                                op=mybir.AluOpType.max,
                            )
                    rn_t = small.tile([128, 64], f32, tag="rn")
                    nc.sync.dma_start(rn_t[:], rn[aslc, :])
                    # sel = P - 10*rnear; sel[win] = max(sel[win], Q); negInner = max_r sel
                    sel = small.tile([128, 64], f32, tag="sel")
                    nc.vector.tensor_sub(sel[:], p_t[:], rn_t[:])
                    nc.vector.tensor_max(
                        sel[:, wlo:whi], sel[:, wlo:whi], q_t[:]
                    )
                    nc.vector.reduce_max(
                        out_s[:, t : t + 1], sel[:], axis=mybir.AxisListType.X
                    )

            nc.sync.dma_start(out2[:], out2_s[:])
            nc.sync.dma_start(out1[:], out1_s[:])

    return nc


def _assign_slots(rv):
    """Bucket anchors by grid row into NT primary tiles (rows [4t, 4t+4))
    plus NSPILL overflow tiles.  Returns (perm [NSLOT], valid [NSLOT])."""
    spill = []
    perm = np.zeros(NSLOT, dtype=np.int64)
    valid = np.zeros(NSLOT, dtype=bool)
    for t in range(NT):
        b = np.where((rv >= 4 * t) & (rv < 4 * t + 4))[0]
        take = b[:128]
        spill.extend(b[128:].tolist())
        perm[t * 128 : t * 128 + len(take)] = take
        valid[t * 128 : t * 128 + len(take)] = True
        if len(take) < 128 and len(take) > 0:
            perm[t * 128 + len(take) : (t + 1) * 128] = take[0]
    if len(spill) > NSPILL * 128:
        raise RuntimeError(f"row-bucket spill overflow: {len(spill)}")
    s0 = NT * 128
    perm[s0 : s0 + len(spill)] = spill
    valid[s0 : s0 + len(spill)] = True
    return perm, valid


def _prep_image(f1, f2, idv, r2v, c2v):
    """Host-side index/mask prep for one image."""
    f1 = f1.reshape(C, HW)
    f2 = f2.reshape(C, HW)
    n1 = np.sqrt((f1 * f1).sum(axis=0))
    f1n = f1 / np.maximum(n1, 1e-12)
    n2 = np.sqrt((f2 * f2).sum(axis=0))
    f2n = f2 / np.maximum(n2, 1e-12)

    r1v = idv // W
    c1v = idv % W
    lin2 = r2v * W + c2v

    d1n = f1n[:, idv]  # [C, N]
    d2n = f2n[:, lin2]  # [C, N]
    pos_inner = (d1n * d2n).sum(axis=0)  # [N]

    perm2, valid2 = _assign_slots(r2v)
    perm1, valid1 = _assign_slots(r1v)

    w = np.arange(64)
    c2p = c2v[perm2]
    c1p = c1v[perm1]
    cn2 = -5.0 * (np.abs(w[:, None] - c2p[None, :]) <= SAFE)  # [64, NSLOT]
    cn1 = -5.0 * (np.abs(w[:, None] - c1p[None, :]) <= SAFE)
    rn2 = 10.0 * (np.abs(w[None, :] - r2v[perm2][:, None]) <= SAFE)  # [NSLOT, 64]
    rn1 = 10.0 * (np.abs(w[None, :] - r1v[perm1][:, None]) <= SAFE)

    onehot = np.tile(np.eye(64, dtype=np.float32), (1, HW // 64))  # [64, HW]

    if MM_DTYPE == "bfloat16":
        from ml_dtypes import bfloat16 as mmdt
    else:
        mmdt = np.float32
    f32 = np.float32
    return {
        "a2": np.concatenate([d1n[:, perm2], cn2], axis=0).astype(mmdt),
        "r2": np.concatenate([f2n, onehot], axis=0).astype(mmdt),
        "a1": np.concatenate([d2n[:, perm1], cn1], axis=0).astype(mmdt),
        "r1": np.concatenate([f1n, onehot], axis=0).astype(mmdt),
        "rn2": rn2.astype(f32),
        "rn1": rn1.astype(f32),
    }, pos_inner.astype(f32), (perm2, valid2, perm1, valid1)


def kernel(x1_encoded, x2_encoded, ids, fmap_pos2, trace=False):
    global LAST_EXEC_NS
    from concourse.bass_utils import run_bass_kernel_spmd

    x1 = np.asarray(x1_encoded, dtype=np.float32)
    x2 = np.asarray(x2_encoded, dtype=np.float32)
    idsv = np.asarray(ids)
    pos2 = np.asarray(fmap_pos2)

    in_maps = []
    pos_inner = []
    perms = []
    for b in range(B):
        m, pi, pv = _prep_image(
            x1[b], x2[b], idsv[b].astype(np.int64),
            pos2[b, 0].astype(np.int64), pos2[b, 1].astype(np.int64),
        )
        in_maps.append(m)
        pos_inner.append(pi)
        perms.append(pv)

    if "nc" not in _COMPILED:
        nc = _build_program()
        _split_multi_waits(nc)
        _COMPILED["nc"] = nc
    nc = _COMPILED["nc"]

    if trace:
        _install_profile_hook()
    res = run_bass_kernel_spmd(
        nc, in_maps, core_ids=list(range(B)), trace=trace
    )
    if trace:
        LAST_EXEC_NS = res.exec_time_ns

    per_image = np.empty(B, dtype=np.float32)
    for b in range(B):
        perm2, valid2, perm1, valid1 = perms[b]
        v2 = res.results[b]["out2"].T.reshape(-1)  # [NSLOT] (slot = t*128 + p)
        v1 = res.results[b]["out1"].T.reshape(-1)
        neg_in2 = np.empty(N, dtype=np.float32)
        neg_in1 = np.empty(N, dtype=np.float32)
        neg_in2[perm2[valid2]] = v2[valid2]
        neg_in1[perm1[valid1]] = v1[valid1]
        max_inner = np.maximum(neg_in1, neg_in2)
        loss_n = np.maximum(1.0 - 2.0 * pos_inner[b] + 2.0 * max_inner, 0.0)
        per_image[b] = loss_n.mean(dtype=np.float64)
    return np.array(per_image.mean(dtype=np.float64), dtype=np.float32)


def _install_profile_hook():
    """antenv.axon_hooks is absent on this image; synthesize it so
    run_bass_kernel_spmd(trace=True) can capture NTFF profiles."""
    import sys
    import types

    if "antenv.axon_hooks" in sys.modules:
        return
    mod = types.ModuleType("antenv.axon_hooks")
    mod._hook = None
    mod.set_axon_ntff_profile_hook = lambda h: setattr(mod, "_hook", h)
    mod.get_axon_ntff_profile_hook = lambda: mod._hook
    sys.modules["antenv.axon_hooks"] = mod
    try:
        import antenv

        antenv.axon_hooks = mod
        from trn_agent_boot.trn_boot import _ntff_profile_via_ctypes

        hook = _ntff_profile_via_ctypes("/opt/axon/libaxon_pjrt.so")
        if hook is not None:
            mod.set_axon_ntff_profile_hook(hook)
    except Exception:
        pass


# revision 24
# speedup vs baseline: 1.2715x; 1.0869x over previous
"""Trainium2 Bass kernel for nn_CorrespondenceLoss.

Correspondence (hinge-margin descriptor) loss over B=8 images, data-parallel
across 8 NeuronCores (one image per core).

Per image (C=64 channels, H=W=64 grid, N=2048 correspondences):
  d1_all = normalize(f1.reshape(C, HW));  d2_all = normalize(f2.reshape(C, HW))
  d1 = d1_all[:, ids]; d2 = d2_all[:, lin(pos2)]
  positive[n] = 2 - 2 * <d1_n, d2_n>
  neg2[n] = min_m (2 - 2*<d1_n, d2_all_m> + 10*[cheb(pos2_n, m) <= 4])
  neg1[n] = min_m (2 - 2*<d2_n, d1_all_m> + 10*[cheb(pos1_n, m) <= 4])
  loss = mean relu(1 + positive - min(neg1, neg2))

Device strategy per image ("matrix" = one of the two N x HW distance matrices):
  The masked min over m is computed as a masked max over inner products.
  The Chebyshev ball is a row-window x col-window rectangle.  The column
  window is folded INTO the matmul with augmented contraction channels:
    innerQ[n, m] = <d1_n, d2_m> - 5 * [|c(m) - c_n| <= 4]
  via lhsT rows 64:128 = -5*cnear^T and rhs rows 64:128 = tile(I64, HW/64)
  (a -5 shift pushes any col-masked entry below every unmasked entry, since
  inner products of unit vectors lie in [-1, 1]).
  Per (anchor n, grid row r):
    P[n,r] = max_c innerP   (plain, K=64 matmul)
    Q[n,r] = max_c innerQ   (col-masked, K=128 matmul)
  and the row-window select is R = max(Q, P - 10*rnear[n,r]), then
  negInner[n] = max_r R.  Host combines:  loss_n = relu(1 - 2*posInner + 2*max(negInner1, negInner2)).

Host does only O(C*HW + N) prep: normalization scales, gathers by index,
mask/one-hot construction, and the final O(N) hinge+mean.
"""

import numpy as np

C = 64
H = 64
W = 64
HW = H * W
N = 2048
B = 8
NT = N // 128  # 16 primary anchor tiles per image (row-bucketed)
NSPILL = 1  # spill tiles for row-bucket overflow (_assign_slots checks the fit)
NT2 = NT + NSPILL
NSLOT = NT2 * 128
MJ = HW // 512  # 8 matmul column blocks
SAFE = 4


def _tile_window(t):
    """Static grid-row window covering every safe-radius band of anchors
    whose row lies in bucket [4t, 4t+4)."""
    wlo = max(0, 4 * t - SAFE)
    whi = min(H, 4 * t + 4 + SAFE)
    return wlo, whi

_COMPILED = {}
LAST_EXEC_NS = None


# ---------------------------------------------------------------------------
# walrus in this environment accepts at most ONE sync-wait per instruction;
# Tile emits instructions with several.  Hoist extras onto NoOps inserted
# just before the over-subscribed instruction (same engine, so program order
# and the wait semantics are preserved).
# ---------------------------------------------------------------------------
def _split_multi_waits(nc, limit=1):
    import bass_rust
    from concourse import mybir

    ctr = 0
    for fn in nc.m.functions:
        for bb in fn.blocks:
            new = []
            for inst in bb.instructions:
                si = inst.sync_info
                if si is not None and len(si.on_wait) > limit:
                    waits = list(si.on_wait)
                    sem = [w for w in waits if w.sync_type == "semaphore"]
                    other = [w for w in waits if w.sync_type != "semaphore"]
                    keep_budget = max(0, limit - len(other))
                    move = sem[:-keep_budget] if keep_budget > 0 else sem
                    keep = other + (sem[-keep_budget:] if keep_budget > 0 else [])
                    if len(keep) > limit:
                        raise RuntimeError(
                            f"cannot split waits on {inst.name}: "
                            f"{len(other)} non-semaphore waits"
                        )
                    for w in move:
                        ctr += 1
                        new.append(
                            mybir.InstNoOp(
                                name=f"WSPLIT-{ctr}",
                                engine=inst.engine,
                                sync_info=bass_rust.SyncInfo(
                                    on_wait=[w], on_update=[]
                                ),
                            )
                        )
                    inst.sync_info = bass_rust.SyncInfo(
                        on_wait=keep, on_update=list(si.on_update)
                    )
                new.append(inst)
            bb.instructions = new
    return ctr


MM_DTYPE = "bfloat16"  # matmul operand dtype: "float32" or "bfloat16"


def _build_program():
    import concourse.bass as bass
    import concourse.tile as tile
    from concourse import mybir

    f32 = mybir.dt.float32
    mmdt = getattr(mybir.dt, MM_DTYPE)
    nc = bass.Bass()

    a2 = nc.dram_tensor("a2", [128, NSLOT], mmdt, kind="ExternalInput")
    r2 = nc.dram_tensor("r2", [128, HW], mmdt, kind="ExternalInput")
    a1 = nc.dram_tensor("a1", [128, NSLOT], mmdt, kind="ExternalInput")
    r1 = nc.dram_tensor("r1", [128, HW], mmdt, kind="ExternalInput")
    rn2 = nc.dram_tensor("rn2", [NSLOT, 64], f32, kind="ExternalInput")
    rn1 = nc.dram_tensor("rn1", [NSLOT, 64], f32, kind="ExternalInput")
    out2 = nc.dram_tensor("out2", [128, NT2], f32, kind="ExternalOutput")
    out1 = nc.dram_tensor("out1", [128, NT2], f32, kind="ExternalOutput")

    bf16 = mybir.dt.bfloat16

    with tile.TileContext(nc) as tc:
        with (
            tc.tile_pool(name="singles", bufs=1) as singles,
            tc.tile_pool(name="small", bufs=4) as small,
            tc.tile_pool(name="tree", bufs=3) as tree,
            tc.tile_pool(name="outp", bufs=1) as outp,
            tc.tile_pool(name="ps", bufs=2, space="PSUM") as psum,
        ):

            def reduce_chunk_act(ps_t, dst):
                """max over innermost 64 of a [128, 2048] PSUM chunk ->
                dst [128, 32], splitting work ACT (cast) + DVE (2x tree)."""
                cast = tree.tile([128, 32, 64], bf16, tag="cast")
                nc.scalar.copy(
                    cast[:], ps_t[:].rearrange("p (r c) -> p r c", c=64)
                )
                t1 = tree.tile([128, 32, 32], bf16, tag="t1")
                nc.vector.tensor_max(t1[:], cast[:, :, 0:32], cast[:, :, 32:64])
                t2 = tree.tile([128, 32, 16], bf16, tag="t2")
                nc.vector.tensor_max(t2[:], t1[:, :, 0:16], t1[:, :, 16:32])
                t3 = tree.tile([128, 32, 8], bf16, tag="t3")
                nc.vector.tensor_max(t3[:], t2[:, :, 0:8], t2[:, :, 8:16])
                t4 = tree.tile([128, 32, 4], bf16, tag="t4")
                nc.vector.tensor_max(t4[:], t3[:, :, 0:4], t3[:, :, 4:8])
                nc.vector.tensor_reduce(
                    dst,
                    t4[:],
                    axis=mybir.AxisListType.X,
                    op=mybir.AluOpType.max,
                )
            a2_s = singles.tile([128, NSLOT], mmdt)
            r2_s = singles.tile([128, HW], mmdt)
            a1_s = singles.tile([128, NSLOT], mmdt)
            r1_s = singles.tile([128, HW], mmdt)
            nc.sync.dma_start(a2_s[:], a2[:])
            nc.sync.dma_start(r2_s[:], r2[:])
            nc.sync.dma_start(a1_s[:], a1[:])
            nc.sync.dma_start(r1_s[:], r1[:])
            # anchor/target duplicates in partitions 64:128 so pairs of K=64
            # P-matmuls can row-pack the PE array (tile_position rows 0/64)
            a2_d = singles.tile([128, NSLOT], mmdt)
            r2_d = singles.tile([128, HW], mmdt)
            a1_d = singles.tile([128, NSLOT], mmdt)
            r1_d = singles.tile([128, HW], mmdt)
            for dup, src in ((a2_d, a2), (r2_d, r2), (a1_d, a1), (r1_d, r1)):
                nc.sync.dma_start(dup[0:64, :], src[0:64, :])
                nc.sync.dma_start(dup[64:128, :], src[0:64, :])
            out2_s = outp.tile([128, NT2], f32)
            out1_s = outp.tile([128, NT2], f32)

            for a_s, r_s, a_d, r_d, rn, out_s in (
                (a2_s, r2_s, a2_d, r2_d, rn2, out2_s),
                (a1_s, r1_s, a1_d, r1_d, rn1, out1_s),
            ):
                for t in range(NT2):
                    aslc = slice(t * 128, (t + 1) * 128)
                    p_t = small.tile([128, 64], f32, tag="p")
                    # P variant (plain, K=64) over the full grid; pairs of
                    # blocks run concurrently in PE rows 0:64 / 64:128
                    for h in range(2):  # m-halves; half h covers r in [32h, 32h+32)
                        ps_t = psum.tile([128, HW // 2], f32, tag="ps")
                        for j in range(MJ // 2):
                            base = 64 * (j % 2)
                            mslc = slice(
                                h * (HW // 2) + j * 512,
                                h * (HW // 2) + (j + 1) * 512,
                            )
                            nc.tensor.matmul(
                                ps_t[:, j * 512 : (j + 1) * 512],
                                a_d[base : base + 64, aslc],
                                r_d[base : base + 64, mslc],
                                start=True,
                                stop=True,
                            )
                        reduce_chunk_act(ps_t, p_t[:, h * 32 : (h + 1) * 32])
                    # Q variant (col-masked, K=128): primary tiles only need
                    # the static 12-grid-row window; spill tiles need all 64.
                    if t < NT:
                        wlo, whi = _tile_window(t)
                    else:
                        wlo, whi = 0, H
                    q_t = small.tile([128, whi - wlo], f32, tag="q")
                    ncols = (whi - wlo) * 64
                    for q0 in range(0, ncols, 2048):
                        qw = min(2048, ncols - q0)
                        ps_q = psum.tile([128, qw], f32, tag="ps")
                        for j in range(0, qw, 512):
                            jw = min(512, qw - j)
                            mslc = slice(
                                wlo * 64 + q0 + j, wlo * 64 + q0 + j + jw
                            )
                            nc.tensor.matmul(
                                ps_q[:, j : j + jw],
                                a_s[:, aslc],
                                r_s[:, mslc],
                                start=True,
                                stop=True,
                            )
                        if qw == 2048:
                            reduce_chunk_act(
                                ps_q, q_t[:, q0 // 64 : (q0 + qw) // 64]
                            )
                        else:
                            nc.vector.tensor_reduce(
                                q_t[:, q0 // 64 : (q0 + qw) // 64],
                                ps_q[:].rearrange("p (r c) -> p r c", c=64),
                                axis=mybir.AxisListType.X,
                                op=mybir.AluOpType.max,
                            )
                    rn_t = small.tile([128, 64], f32, tag="rn")
                    nc.sync.dma_start(rn_t[:], rn[aslc, :])
                    # sel = P - 10*rnear; sel[win] = max(sel[win], Q); negInner = max_r sel
                    sel = small.tile([128, 64], f32, tag="sel")
                    nc.vector.tensor_sub(sel[:], p_t[:], rn_t[:])
                    nc.vector.tensor_max(
                        sel[:, wlo:whi], sel[:, wlo:whi], q_t[:]
                    )
                    nc.vector.reduce_max(
                        out_s[:, t : t + 1], sel[:], axis=mybir.AxisListType.X
                    )

            nc.sync.dma_start(out2[:], out2_s[:])
            nc.sync.dma_start(out1[:], out1_s[:])

    return nc


def _assign_slots(rv):
    """Bucket anchors by grid row into NT primary tiles (rows [4t, 4t+4))
    plus NSPILL overflow tiles.  Returns (perm [NSLOT], valid [NSLOT])."""
    spill = []
    perm = np.zeros(NSLOT, dtype=np.int64)
    valid = np.zeros(NSLOT, dtype=bool)
    for t in range(NT):
        b = np.where((rv >= 4 * t) & (rv < 4 * t + 4))[0]
        take = b[:128]
        spill.extend(b[128:].tolist())
        perm[t * 128 : t * 128 + len(take)] = take
        valid[t * 128 : t * 128 + len(take)] = True
        if len(take) < 128 and len(take) > 0:
            perm[t * 128 + len(take) : (t + 1) * 128] = take[0]
    if len(spill) > NSPILL * 128:
        raise RuntimeError(f"row-bucket spill overflow: {len(spill)}")
    s0 = NT * 128
    perm[s0 : s0 + len(spill)] = spill
    valid[s0 : s0 + len(spill)] = True
    return perm, valid


def _prep_image(f1, f2, idv, r2v, c2v):
    """Host-side index/mask prep for one image."""
    f1 = f1.reshape(C, HW)
    f2 = f2.reshape(C, HW)
    n1 = np.sqrt((f1 * f1).sum(axis=0))
    f1n = f1 / np.maximum(n1, 1e-12)
    n2 = np.sqrt((f2 * f2).sum(axis=0))
    f2n = f2 / np.maximum(n2, 1e-12)

    r1v = idv // W
    c1v = idv % W
    lin2 = r2v * W + c2v

    d1n = f1n[:, idv]  # [C, N]
    d2n = f2n[:, lin2]  # [C, N]
    pos_inner = (d1n * d2n).sum(axis=0)  # [N]

    perm2, valid2 = _assign_slots(r2v)
    perm1, valid1 = _assign_slots(r1v)

    w = np.arange(64)
    c2p = c2v[perm2]
    c1p = c1v[perm1]
    cn2 = -5.0 * (np.abs(w[:, None] - c2p[None, :]) <= SAFE)  # [64, NSLOT]
    cn1 = -5.0 * (np.abs(w[:, None] - c1p[None, :]) <= SAFE)
    rn2 = 10.0 * (np.abs(w[None, :] - r2v[perm2][:, None]) <= SAFE)  # [NSLOT, 64]
    rn1 = 10.0 * (np.abs(w[None, :] - r1v[perm1][:, None]) <= SAFE)

    onehot = np.tile(np.eye(64, dtype=np.float32), (1, HW // 64))  # [64, HW]

    if MM_DTYPE == "bfloat16":
        from ml_dtypes import bfloat16 as mmdt
    else:
        mmdt = np.float32
    f32 = np.float32
    return {
        "a2": np.concatenate([d1n[:, perm2], cn2], axis=0).astype(mmdt),
        "r2": np.concatenate([f2n, onehot], axis=0).astype(mmdt),
        "a1": np.concatenate([d2n[:, perm1], cn1], axis=0).astype(mmdt),
        "r1": np.concatenate([f1n, onehot], axis=0).astype(mmdt),
        "rn2": rn2.astype(f32),
        "rn1": rn1.astype(f32),
    }, pos_inner.astype(f32), (perm2, valid2, perm1, valid1)


def kernel(x1_encoded, x2_encoded, ids, fmap_pos2, trace=False):
    global LAST_EXEC_NS
    from concourse.bass_utils import run_bass_kernel_spmd

    x1 = np.asarray(x1_encoded, dtype=np.float32)
    x2 = np.asarray(x2_encoded, dtype=np.float32)
    idsv = np.asarray(ids)
    pos2 = np.asarray(fmap_pos2)

    in_maps = []
    pos_inner = []
    perms = []
    for b in range(B):
        m, pi, pv = _prep_image(
            x1[b], x2[b], idsv[b].astype(np.int64),
            pos2[b, 0].astype(np.int64), pos2[b, 1].astype(np.int64),
        )
        in_maps.append(m)
        pos_inner.append(pi)
        perms.append(pv)

    if "nc" not in _COMPILED:
        nc = _build_program()
        _split_multi_waits(nc)
        _COMPILED["nc"] = nc
    nc = _COMPILED["nc"]

    if trace:
        _install_profile_hook()
    res = run_bass_kernel_spmd(
        nc, in_maps, core_ids=list(range(B)), trace=trace
    )
    if trace:
        LAST_EXEC_NS = res.exec_time_ns

    per_image = np.empty(B, dtype=np.float32)
    for b in range(B):
        perm2, valid2, perm1, valid1 = perms[b]
        v2 = res.results[b]["out2"].T.reshape(-1)  # [NSLOT] (slot = t*128 + p)
        v1 = res.results[b]["out1"].T.reshape(-1)
        neg_in2 = np.empty(N, dtype=np.float32)
        neg_in1 = np.empty(N, dtype=np.float32)
        neg_in2[perm2[valid2]] = v2[valid2]
        neg_in1[perm1[valid1]] = v1[valid1]
        max_inner = np.maximum(neg_in1, neg_in2)
        loss_n = np.maximum(1.0 - 2.0 * pos_inner[b] + 2.0 * max_inner, 0.0)
        per_image[b] = loss_n.mean(dtype=np.float64)
    return np.array(per_image.mean(dtype=np.float64), dtype=np.float32)


def _install_profile_hook():
    """antenv.axon_hooks is absent on this image; synthesize it so
    run_bass_kernel_spmd(trace=True) can capture NTFF profiles."""
    import sys
    import types

    if "antenv.axon_hooks" in sys.modules:
        return
    mod = types.ModuleType("antenv.axon_hooks")
    mod._hook = None
    mod.set_axon_ntff_profile_hook = lambda h: setattr(mod, "_hook", h)
    mod.get_axon_ntff_profile_hook = lambda: mod._hook
    sys.modules["antenv.axon_hooks"] = mod
    try:
        import antenv

        antenv.axon_hooks = mod
        from trn_agent_boot.trn_boot import _ntff_profile_via_ctypes

        hook = _ntff_profile_via_ctypes("/opt/axon/libaxon_pjrt.so")
        if hook is not None:
            mod.set_axon_ntff_profile_hook(hook)
    except Exception:
        pass
